# revision 22
# baseline (speedup 1.0000x reference)
"""MoLoRA (mixture of LoRA experts with top-2 routing) Trainium2 Bass kernel.

Math (per token t, hidden H=640, experts E=5, rank R=8, router hidden 256):
  h      = silu(x @ W1 + b1)                 [T, 256]
  logits = h @ W2 + b2                       [T, 5]
  top-2 of softmax(logits), renormalized  == softmax over the top-2 logits:
     w[t, e] = sigmoid(2*l_e - m1 - m2) * [l_e >= m2]   (m1/m2 = top-2 logits)
  low    = x @ Acat                          [T, 40]   (Acat[h,(e,r)] = A[e,h,r])
  delta  = (low * w_expanded) @ (Bcat * 2)   [T, 640]  (Bcat[(e,r),h] = Bm[e,r,h])
  out    = base_output + delta

mm1/low run in float32r (fast-fp32 PE mode, full rate at N>=256) off exact
fp32 PE transposes of x; mm2 is exact fp32 (tiny); the delta path is bf16.
Tiles are 512 tokens to amortize per-instruction overhead.  Sharding:
data-parallel over 8 NeuronCores (4096 tokens each), params replicated.
"""

import numpy as np
from contextlib import ExitStack

import concourse.bass as bass
import concourse.tile as tile
from concourse import bacc
from concourse import mybir
from concourse.bass import ts
from concourse.masks import make_identity
from concourse.bass_utils import run_bass_kernel_spmd

F32 = mybir.dt.float32
F32R = mybir.dt.float32r
BF16 = mybir.dt.bfloat16
AF = mybir.ActivationFunctionType
ALU = mybir.AluOpType
AX = mybir.AxisListType

H = 640          # hidden
E = 5            # experts
R = 8            # lora rank
ER = E * R       # 40
RH = 256         # router hidden
HC = H // 128    # 5 h-chunks
RC = RH // 128   # 2 router-hidden chunks
SCALING = 16.0 / R
N_CORES = 8
T_CORE = 4096    # tokens per core (32768 / 8)
TT = 512         # token tile (4 j-halves of 128)
JJ = TT // 128   # 4

# delta is computed in 5 PSUM-bank-aligned 512-column chunks of the flat
# (j, h) output: chunk -> list of (j, h0, h1, dl_offset)
DELTA_CHUNKS = []
for _c5 in range(5):
    _g0, _g1 = _c5 * 512, (_c5 + 1) * 512
    _parts = []
    for _j in range(JJ):
        _a, _b = max(_g0, _j * H), min(_g1, (_j + 1) * H)
        if _a < _b:
            _parts.append((_j, _a - _j * H, _b - _j * H, _a - _g0))
    DELTA_CHUNKS.append(_parts)


def build_kernel(t_core=T_CORE, niter=1, timing_mode=False):
    assert t_core % TT == 0
    ntiles = t_core // TT
    nc = bacc.Bacc()

    if timing_mode:
        # big tensors stay on-device (uninitialized DRAM) so per-call wall
        # time isn't dominated by the axon host transfer; HBM traffic is
        # identical to the real kernel.
        x_d = nc.dram_tensor("x_int", [t_core, H], F32)[:, :]
        base_d = nc.dram_tensor("base_int", [t_core, H], F32)[:, :]
        out_d = nc.dram_tensor("out_int", [t_core, H], F32)[:, :]
        dummy_d = nc.declare_dram_parameter("dummy_out", [1, 4], F32, isOutput=True)
    else:
        x_d = nc.declare_dram_parameter("x", [t_core, H], F32, isOutput=False)
        base_d = nc.declare_dram_parameter("base", [t_core, H], F32, isOutput=False)
        out_d = nc.declare_dram_parameter("out", [t_core, H], F32, isOutput=True)
        dummy_d = None
    w1_d = nc.declare_dram_parameter("W1", [H, RH], F32, isOutput=False)
    b1_d = nc.declare_dram_parameter("b1", [RH], F32, isOutput=False)
    w2_d = nc.declare_dram_parameter("W2", [RH, E], F32, isOutput=False)
    b2_d = nc.declare_dram_parameter("b2", [E], F32, isOutput=False)
    a_d = nc.declare_dram_parameter("A", [E, H, R], F32, isOutput=False)
    bm_d = nc.declare_dram_parameter("Bm", [E, R, H], F32, isOutput=False)

    with ExitStack() as ctx:
        tc = ctx.enter_context(tile.TileContext(nc))
        const = ctx.enter_context(tc.tile_pool(name="const", bufs=1))
        xin_p = ctx.enter_context(tc.tile_pool(name="xin", bufs=2))
        bout_p = ctx.enter_context(tc.tile_pool(name="bout", bufs=2))
        xt_p = ctx.enter_context(tc.tile_pool(name="xt", bufs=2))
        ht_p = ctx.enter_context(tc.tile_pool(name="ht", bufs=2))
        small_p = ctx.enter_context(tc.tile_pool(name="small", bufs=2))
        lw_p = ctx.enter_context(tc.tile_pool(name="lw", bufs=2))
        # PSUM budget (8 banks of 2KB):
        #  ps_xtp [128, 512] f32 bufs=2                  -> 2 banks
        #  ps_h   [128, 2, 512] f32 bufs=1               -> 2 banks
        #  ps_lo  [40, 512] f32 bufs=1                   -> 1 bank
        #  ps_wl  [128, 512] f32 bufs=1 (lg + wrt bf16)  -> 1 bank
        #  ps_dl  [128, 512] f32 bufs=2                  -> 2 banks
        ps_xtp = ctx.enter_context(tc.tile_pool(name="ps_xtp", bufs=2, space="PSUM"))
        ps_h = ctx.enter_context(tc.tile_pool(name="ps_h", bufs=1, space="PSUM"))
        ps_lo = ctx.enter_context(tc.tile_pool(name="ps_lo", bufs=1, space="PSUM"))
        ps_wl = ctx.enter_context(tc.tile_pool(name="ps_wl", bufs=1, space="PSUM"))
        ps_dl = ctx.enter_context(tc.tile_pool(name="ps_dl", bufs=2, space="PSUM"))

        # ---- constants / replicated params ----
        ident = const.tile([128, 128], F32)
        make_identity(nc, ident)
        ident_bf = const.tile([128, 128], BF16)
        nc.vector.tensor_copy(out=ident_bf, in_=ident)

        w1_sb = const.tile([128, HC, RH], F32)
        nc.gpsimd.dma_start(out=w1_sb, in_=w1_d.rearrange("(c p) m -> p c m", p=128))
        w1_r = const.tile([128, HC, RH], F32R)
        nc.vector.tensor_copy(out=w1_r, in_=w1_sb)
        b1_sb = const.tile([128, RC], F32)
        nc.gpsimd.dma_start(out=b1_sb, in_=b1_d.rearrange("(c p) -> p c", p=128))
        w2_sb = const.tile([128, RC, E], F32)
        nc.gpsimd.dma_start(out=w2_sb, in_=w2_d.rearrange("(c p) e -> p c e", p=128))
        # b2 replicated to all partitions (added on DVE, not via PE)
        b2_rep = const.tile([128, E], F32)
        nc.gpsimd.dma_start(
            out=b2_rep, in_=b2_d[:].unsqueeze(0).to_broadcast((128, E))
        )
        # LoRA params concatenated over (e, r): index m = e*R + r.
        acat_sb = const.tile([128, HC, E, R], F32)
        for e in range(E):
            for c in range(HC):
                nc.gpsimd.dma_start(
                    out=acat_sb[:, c, e, :],
                    in_=a_d[e, c * 128 : (c + 1) * 128, :],
                )
        acat_r = const.tile([128, HC, E, R], F32R)
        nc.vector.tensor_copy(out=acat_r, in_=acat_sb)
        bcat_sb = const.tile([ER, H], F32)
        for e in range(E):
            nc.gpsimd.dma_start(out=bcat_sb[e * R : (e + 1) * R, :], in_=bm_d[e, :, :])
        # LoRA SCALING (=2.0) folded into Bcat here; delta path is bf16.
        bcat_bf = const.tile([ER, H], BF16)
        nc.vector.tensor_scalar(
            out=bcat_bf, in0=bcat_sb, scalar1=float(SCALING), scalar2=None,
            op0=ALU.mult,
        )

        if dummy_d is not None:
            dnm = const.tile([1, 4], F32)
            nc.vector.memset(dnm, 1.0)
            nc.sync.dma_start(out=dummy_d[:, :], in_=dnm)

        loop_ctx = tc.For_i(0, niter, 1) if niter > 1 else None
        if loop_ctx is not None:
            ctx.enter_context(loop_ctx)

        def emit_loads(ip):
            """one 1024-token (2-tile) load pair on SP/ACT HWDGE"""
            tok = ip * 2 * TT
            x2 = xin_p.tile([128, 2 * JJ, H], F32)
            nc.sync.dma_start(
                out=x2,
                in_=x_d[tok : tok + 2 * TT, :].rearrange("(j p) h -> p j h", p=128),
            )
            bo2 = bout_p.tile([128, 2 * JJ, H], F32)
            with tc.high_priority():
                nc.scalar.dma_start(
                    out=bo2,
                    in_=base_d[tok : tok + 2 * TT, :].rearrange(
                        "(j p) h -> p j h", p=128
                    ),
                )
            return x2, bo2

        def emit_front(i, x2, bo2):
            """xT transposes (exact fp32) + f32r copy, c-chunk rotated"""
            tok = i * TT
            half = i % 2
            x_nat = x2[:, half * JJ : (half + 1) * JJ, :]
            bo = bo2[:, half * JJ : (half + 1) * JJ, :]
            xt_r = xt_p.tile([128, HC, TT], F32R)
            for c in range(HC):
                xtp = ps_xtp.tile([128, TT], F32, tag="xtp")
                for tj in range(JJ):
                    nc.tensor.transpose(
                        out=xtp[:, ts(tj, 128)],
                        in_=x_nat[:, tj, ts(c, 128)],
                        identity=ident,
                    )
                with tc.high_priority():
                    nc.scalar.copy(out=xt_r[:, c, :], in_=xtp)
            return {"bo": bo, "bo2": bo2, "half": half, "xt_r": xt_r, "tok": tok}

        def emit_router(st):
            """mm1 -> silu -> (low interleaved) -> mm2"""
            xt_r = st["xt_r"]
            h_ps = ps_h.tile([128, RC, TT], F32, tag="h")
            for c2 in range(RC):
                for c in range(HC):
                    nc.tensor.matmul(
                        out=h_ps[:, c2, :],
                        lhsT=w1_r[:, c, ts(c2, 128)],
                        rhs=xt_r[:, c, :],
                        start=(c == 0),
                        stop=(c == HC - 1),
                    )
            # silu(z) = z * sigmoid(z), z = h + b1: ACT computes sigmoid and
            # z (identity+bias), Pool multiplies (keeps DVE free)
            sg_sb = ht_p.tile([128, RC, TT], F32, tag="sg")
            hb_sb = ht_p.tile([128, RC, TT], F32, tag="hb")
            ht_sb = ht_p.tile([128, RC, TT], F32, tag="ht")
            for c2 in range(RC):
                nc.scalar.activation(
                    out=sg_sb[:, c2, :], in_=h_ps[:, c2, :],
                    func=AF.Sigmoid, bias=b1_sb[:, c2 : c2 + 1],
                )
                nc.scalar.activation(
                    out=hb_sb[:, c2, :], in_=h_ps[:, c2, :],
                    func=AF.Identity, bias=b1_sb[:, c2 : c2 + 1],
                )
            nc.gpsimd.tensor_tensor(out=ht_sb, in0=hb_sb, in1=sg_sb, op=ALU.mult)

            # lowT[(e,r), t] = (x @ Acat)^T (f32r, N=512)
            low_ps = ps_lo.tile([ER, TT], F32, tag="lo")
            for c in range(HC):
                nc.tensor.matmul(
                    out=low_ps,
                    lhsT=acat_r[:, c, :, :],
                    rhs=xt_r[:, c, :],
                    start=(c == 0),
                    stop=(c == HC - 1),
                )
            # wl bank: lg f32 in [:, 0:20], wrt bf16 in f32-cols [128:384]
            wl = ps_wl.tile([128, 512], F32, tag="wl")
            lg_ps = wl[:, 0 : JJ * E].rearrange("p (j e) -> p j e", j=JJ)
            # router mm2 (token-major logits), exact fp32, b2 added on DVE
            for j in range(JJ):
                for c2 in range(RC):
                    nc.tensor.matmul(
                        out=lg_ps[:, j, :],
                        lhsT=ht_sb[:, c2, ts(j, 128)],
                        rhs=w2_sb[:, c2, :],
                        start=(c2 == 0),
                        stop=(c2 == RC - 1),
                    )
            st["low_ps"], st["wl"], st["lg_ps"] = low_ps, wl, lg_ps

        def emit_weights(st):
            """j-merged top-2 + renormalized weights:
            w[e] = sigmoid(2*lg_e - m1 - m2) * [lg_e >= m2], expanded over r."""
            lg = small_p.tile([128, JJ, E], F32, tag="lg")
            nc.vector.tensor_tensor(
                out=lg, in0=st["lg_ps"],
                in1=b2_rep.unsqueeze(1).to_broadcast((128, JJ, E)), op=ALU.add,
            )
            m1 = small_p.tile([128, JJ], F32, tag="m1")
            nc.vector.tensor_reduce(out=m1, in_=lg, axis=AX.X, op=ALU.max)
            mask1 = small_p.tile([128, JJ, E], F32, tag="mask1")
            nc.vector.tensor_tensor(
                out=mask1, in0=lg, in1=m1.unsqueeze(2).to_broadcast((128, JJ, E)),
                op=ALU.is_equal,
            )
            masked = small_p.tile([128, JJ, E], F32, tag="masked")
            nc.vector.scalar_tensor_tensor(
                out=masked, in0=mask1, scalar=-1e30, in1=lg,
                op0=ALU.mult, op1=ALU.add,
            )
            m2 = small_p.tile([128, JJ], F32, tag="m2")
            nc.vector.tensor_reduce(out=m2, in_=masked, axis=AX.X, op=ALU.max)
            s2 = small_p.tile([128, JJ], F32, tag="s2")
            nc.vector.tensor_tensor(out=s2, in0=m1, in1=m2, op=ALU.add)
            argt = small_p.tile([128, JJ, E], F32, tag="argt")
            nc.vector.scalar_tensor_tensor(
                out=argt, in0=lg, scalar=2.0,
                in1=s2.unsqueeze(2).to_broadcast((128, JJ, E)),
                op0=ALU.mult, op1=ALU.subtract,
            )
            sig = small_p.tile([128, JJ, E], F32, tag="sig")
            nc.scalar.activation(out=sig, in_=argt, func=AF.Sigmoid)
            mge = small_p.tile([128, JJ, E], F32, tag="mge")
            nc.vector.tensor_tensor(
                out=mge, in0=lg, in1=m2.unsqueeze(2).to_broadcast((128, JJ, E)),
                op=ALU.is_ge,
            )
            # fused weight + expansion over r: w_exp[t, j, e, r] = sig*mge
            w_exp = small_p.tile([128, JJ, E, R], BF16, tag="w_exp")
            nc.vector.tensor_tensor(
                out=w_exp,
                in0=sig.unsqueeze(3).to_broadcast((128, JJ, E, R)),
                in1=mge.unsqueeze(3).to_broadcast((128, JJ, E, R)),
                op=ALU.mult,
            )
            st["w_exp"] = w_exp

        def emit_m(st):
            """wT transpose (into wl bank, bf16) + weighted-low"""
            # bf16 view of wl f32-cols [128:384] = 512 bf16 cols
            wrt_ps = st["wl"][:, 128:384].bitcast(BF16)[0:ER, :].rearrange(
                "p (j t) -> p j t", j=JJ
            )
            for j in range(JJ):
                nc.tensor.transpose(
                    out=wrt_ps[:, j, :],
                    in_=st["w_exp"][:, j, :, :].rearrange("p e r -> p (e r)"),
                    identity=ident_bf,
                )
            wrt_sb = lw_p.tile([ER, JJ, 128], BF16, tag="wrt_sb")
            nc.scalar.copy(out=wrt_sb, in_=wrt_ps)
            lw_sb = lw_p.tile([ER, TT], BF16)
            nc.vector.tensor_tensor(
                out=lw_sb,
                in0=st["low_ps"],
                in1=wrt_sb.rearrange("p j t -> p (j t)"),
                op=ALU.mult,
            )
            st["lw_sb"] = lw_sb

        def emit_b(st):
            """back half: delta matmuls (bf16) in 5 bank-aligned chunks of the
            flat (j, h) output, fused PSUM+base adds, store"""
            bo, lw_sb, tok = st["bo"], st["lw_sb"], st["tok"]
            bo_flat = bo.rearrange("p j h -> p (j h)")
            for c5, parts in enumerate(DELTA_CHUNKS):
                dl = ps_dl.tile([128, 512], F32, tag="dl")
                for j, h0, h1, off in parts:
                    nc.tensor.matmul(
                        out=dl[:, off : off + (h1 - h0)],
                        lhsT=lw_sb[:, ts(j, 128)],
                        rhs=bcat_bf[:, h0:h1],
                        start=True, stop=True,
                    )
                nc.vector.tensor_tensor(
                    out=bo_flat[:, c5 * 512 : (c5 + 1) * 512],
                    in0=dl,
                    in1=bo_flat[:, c5 * 512 : (c5 + 1) * 512],
                    op=ALU.add,
                )
            # store on the SWDGE (gpsimd) queue: keeps the waiting store off
            # the HWDGE FIFOs so it can't head-of-line block the next loads
            nc.gpsimd.dma_start(
                out=out_d[tok : tok + TT, :].rearrange("(j p) h -> p j h", p=128),
                in_=bo,
            )

        prev = None
        x2 = bo2 = None
        for i in range(ntiles):
            if i % 2 == 0:
                x2, bo2 = emit_loads(i // 2)
            st = emit_front(i, x2, bo2)
            emit_router(st)
            emit_weights(st)
            emit_m(st)
            if prev is not None:
                emit_b(prev)
            prev = st
        emit_b(prev)

    return nc


_CACHE = {}


def _get_nc(t_core=T_CORE, niter=1, timing_mode=False):
    key = (t_core, niter, timing_mode)
    if key not in _CACHE:
        nc = build_kernel(t_core, niter, timing_mode)
        nc.finalize()
        _CACHE[key] = nc
    return _CACHE[key]


def kernel(x, base_output, W1, b1, W2, b2, A, Bm):
    x = np.ascontiguousarray(np.asarray(x), dtype=np.float32)
    base_output = np.ascontiguousarray(np.asarray(base_output), dtype=np.float32)
    W1 = np.ascontiguousarray(np.asarray(W1), dtype=np.float32)
    b1 = np.ascontiguousarray(np.asarray(b1), dtype=np.float32)
    W2 = np.ascontiguousarray(np.asarray(W2), dtype=np.float32)
    b2 = np.ascontiguousarray(np.asarray(b2), dtype=np.float32)
    A = np.ascontiguousarray(np.asarray(A), dtype=np.float32)
    Bm = np.ascontiguousarray(np.asarray(Bm), dtype=np.float32)

    B, S, _ = x.shape
    assert B * S == N_CORES * T_CORE
    xs = x.reshape(N_CORES, T_CORE, H)
    bs = base_output.reshape(N_CORES, T_CORE, H)

    nc = _get_nc()
    in_maps = [
        {
            "x": np.ascontiguousarray(xs[i]),
            "base": np.ascontiguousarray(bs[i]),
            "W1": W1, "b1": b1, "W2": W2, "b2": b2, "A": A, "Bm": Bm,
        }
        for i in range(N_CORES)
    ]
    res = run_bass_kernel_spmd(nc, in_maps, list(range(N_CORES))).results
    out = np.stack([res[i]["out"] for i in range(N_CORES)], axis=0)
    return out.reshape(B, S, H).astype(np.float32)


# revision 23
# speedup vs baseline: 1.1118x; 1.1118x over previous
"""MoLoRA (mixture of LoRA experts with top-2 routing) Trainium2 Bass kernel.

Math (per token t, hidden H=640, experts E=5, rank R=8, router hidden 256):
  h      = silu(x @ W1 + b1)                 [T, 256]
  logits = h @ W2 + b2                       [T, 5]
  top-2 of softmax(logits), renormalized  == softmax over the top-2 logits:
     w[t, e] = sigmoid(2*l_e - m1 - m2) * [l_e >= m2]   (m1/m2 = top-2 logits)
  low    = x @ Acat                          [T, 40]   (Acat[h,(e,r)] = A[e,h,r])
  delta  = (low * w_expanded) @ (Bcat * 2)   [T, 640]  (Bcat[(e,r),h] = Bm[e,r,h])
  out    = base_output + delta

mm1/low run in float32r (fast-fp32 PE mode, full rate at N>=256) off exact
fp32 PE transposes of x; mm2 is exact fp32 (tiny); the delta path is bf16.
Tiles are 512 tokens to amortize per-instruction overhead.  Sharding:
data-parallel over 8 NeuronCores (4096 tokens each), params replicated.
"""

import numpy as np
from contextlib import ExitStack

import concourse.bass as bass
import concourse.tile as tile
from concourse import bacc
from concourse import mybir
from concourse.bass import ts
from concourse.masks import make_identity
from concourse.bass_utils import run_bass_kernel_spmd

F32 = mybir.dt.float32
F32R = mybir.dt.float32r
BF16 = mybir.dt.bfloat16
AF = mybir.ActivationFunctionType
ALU = mybir.AluOpType
AX = mybir.AxisListType

H = 640          # hidden
E = 5            # experts
R = 8            # lora rank
ER = E * R       # 40
RH = 256         # router hidden
HC = H // 128    # 5 h-chunks
RC = RH // 128   # 2 router-hidden chunks
SCALING = 16.0 / R
N_CORES = 8
T_CORE = 4096    # tokens per core (32768 / 8)
TT = 512         # token tile (4 j-halves of 128)
JJ = TT // 128   # 4

# delta is computed in 5 PSUM-bank-aligned 512-column chunks of the flat
# (j, h) output: chunk -> list of (j, h0, h1, dl_offset)
DELTA_CHUNKS = []
for _c5 in range(5):
    _g0, _g1 = _c5 * 512, (_c5 + 1) * 512
    _parts = []
    for _j in range(JJ):
        _a, _b = max(_g0, _j * H), min(_g1, (_j + 1) * H)
        if _a < _b:
            _parts.append((_j, _a - _j * H, _b - _j * H, _a - _g0))
    DELTA_CHUNKS.append(_parts)


def build_kernel(t_core=T_CORE, niter=1, timing_mode=False):
    assert t_core % TT == 0
    ntiles = t_core // TT
    nc = bacc.Bacc()

    if timing_mode:
        # big tensors stay on-device (uninitialized DRAM) so per-call wall
        # time isn't dominated by the axon host transfer; HBM traffic is
        # identical to the real kernel.
        x_d = nc.dram_tensor("x_int", [t_core, H], F32)[:, :]
        base_d = nc.dram_tensor("base_int", [t_core, H], F32)[:, :]
        out_d = nc.dram_tensor("out_int", [t_core, H], F32)[:, :]
        dummy_d = nc.declare_dram_parameter("dummy_out", [1, 4], F32, isOutput=True)
    else:
        x_d = nc.declare_dram_parameter("x", [t_core, H], F32, isOutput=False)
        base_d = nc.declare_dram_parameter("base", [t_core, H], F32, isOutput=False)
        out_d = nc.declare_dram_parameter("out", [t_core, H], F32, isOutput=True)
        dummy_d = None
    w1_d = nc.declare_dram_parameter("W1", [H, RH], F32, isOutput=False)
    b1_d = nc.declare_dram_parameter("b1", [RH], F32, isOutput=False)
    w2_d = nc.declare_dram_parameter("W2", [RH, E], F32, isOutput=False)
    b2_d = nc.declare_dram_parameter("b2", [E], F32, isOutput=False)
    a_d = nc.declare_dram_parameter("A", [E, H, R], F32, isOutput=False)
    bm_d = nc.declare_dram_parameter("Bm", [E, R, H], F32, isOutput=False)

    with ExitStack() as ctx:
        tc = ctx.enter_context(tile.TileContext(nc))
        const = ctx.enter_context(tc.tile_pool(name="const", bufs=1))
        xin_p = ctx.enter_context(tc.tile_pool(name="xin", bufs=2))
        bout_p = ctx.enter_context(tc.tile_pool(name="bout", bufs=2))
        xt_p = ctx.enter_context(tc.tile_pool(name="xt", bufs=2))
        ht_p = ctx.enter_context(tc.tile_pool(name="ht", bufs=2))
        small_p = ctx.enter_context(tc.tile_pool(name="small", bufs=2))
        lw_p = ctx.enter_context(tc.tile_pool(name="lw", bufs=2))
        # PSUM budget (8 banks of 2KB):
        #  ps_xtp [128, 512] f32 bufs=2                  -> 2 banks
        #  ps_h   [128, 2, 512] f32 bufs=1               -> 2 banks
        #  ps_lo  [40, 512] f32 bufs=1                   -> 1 bank
        #  ps_wl  [128, 512] f32 bufs=1 (lg + wrt bf16)  -> 1 bank
        #  ps_dl  [128, 512] f32 bufs=2                  -> 2 banks
        ps_xtp = ctx.enter_context(tc.tile_pool(name="ps_xtp", bufs=1, space="PSUM"))
        ps_h = ctx.enter_context(tc.tile_pool(name="ps_h", bufs=1, space="PSUM"))
        ps_lo = ctx.enter_context(tc.tile_pool(name="ps_lo", bufs=1, space="PSUM"))
        ps_wl = ctx.enter_context(tc.tile_pool(name="ps_wl", bufs=1, space="PSUM"))
        ps_dl = ctx.enter_context(tc.tile_pool(name="ps_dl", bufs=3, space="PSUM"))

        # ---- constants / replicated params ----
        ident = const.tile([128, 128], F32)
        make_identity(nc, ident)
        ident_bf = const.tile([128, 128], BF16)
        nc.vector.tensor_copy(out=ident_bf, in_=ident)

        w1_sb = const.tile([128, HC, RH], F32)
        nc.gpsimd.dma_start(out=w1_sb, in_=w1_d.rearrange("(c p) m -> p c m", p=128))
        w1_r = const.tile([128, HC, RH], F32R)
        nc.vector.tensor_copy(out=w1_r, in_=w1_sb)
        b1_sb = const.tile([128, RC], F32)
        nc.gpsimd.dma_start(out=b1_sb, in_=b1_d.rearrange("(c p) -> p c", p=128))
        w2_sb = const.tile([128, RC, E], F32)
        nc.gpsimd.dma_start(out=w2_sb, in_=w2_d.rearrange("(c p) e -> p c e", p=128))
        # b2 replicated to all partitions (added on DVE, not via PE)
        b2_rep = const.tile([128, E], F32)
        nc.gpsimd.dma_start(
            out=b2_rep, in_=b2_d[:].unsqueeze(0).to_broadcast((128, E))
        )
        # LoRA params concatenated over (e, r): index m = e*R + r.
        acat_sb = const.tile([128, HC, E, R], F32)
        for e in range(E):
            for c in range(HC):
                nc.gpsimd.dma_start(
                    out=acat_sb[:, c, e, :],
                    in_=a_d[e, c * 128 : (c + 1) * 128, :],
                )
        acat_r = const.tile([128, HC, E, R], F32R)
        nc.vector.tensor_copy(out=acat_r, in_=acat_sb)
        bcat_sb = const.tile([ER, H], F32)
        for e in range(E):
            nc.gpsimd.dma_start(out=bcat_sb[e * R : (e + 1) * R, :], in_=bm_d[e, :, :])
        # LoRA SCALING (=2.0) folded into Bcat here; delta path is bf16.
        bcat_bf = const.tile([ER, H], BF16)
        nc.vector.tensor_scalar(
            out=bcat_bf, in0=bcat_sb, scalar1=float(SCALING), scalar2=None,
            op0=ALU.mult,
        )

        if dummy_d is not None:
            dnm = const.tile([1, 4], F32)
            nc.vector.memset(dnm, 1.0)
            nc.sync.dma_start(out=dummy_d[:, :], in_=dnm)

        loop_ctx = tc.For_i(0, niter, 1) if niter > 1 else None
        if loop_ctx is not None:
            ctx.enter_context(loop_ctx)

        def emit_loads(ip):
            """one 1024-token (2-tile) load pair on SP/ACT HWDGE"""
            tok = ip * 2 * TT
            x2 = xin_p.tile([128, 2 * JJ, H], F32)
            nc.sync.dma_start(
                out=x2,
                in_=x_d[tok : tok + 2 * TT, :].rearrange("(j p) h -> p j h", p=128),
            )
            bo2 = bout_p.tile([128, 2 * JJ, H], F32)
            with tc.high_priority():
                nc.scalar.dma_start(
                    out=bo2,
                    in_=base_d[tok : tok + 2 * TT, :].rearrange(
                        "(j p) h -> p j h", p=128
                    ),
                )
            return x2, bo2

        def emit_front(i, x2, bo2):
            """xT transposes (exact fp32) + f32r copy, c-chunk rotated"""
            tok = i * TT
            half = i % 2
            x_nat = x2[:, half * JJ : (half + 1) * JJ, :]
            bo = bo2[:, half * JJ : (half + 1) * JJ, :]
            xt_r = xt_p.tile([128, HC, TT], F32R)
            for c in range(HC):
                xtp = ps_xtp.tile([128, TT], F32, tag="xtp")
                for tj in range(JJ):
                    nc.tensor.transpose(
                        out=xtp[:, ts(tj, 128)],
                        in_=x_nat[:, tj, ts(c, 128)],
                        identity=ident,
                    )
                nc.scalar.copy(out=xt_r[:, c, :], in_=xtp)
            return {"bo": bo, "bo2": bo2, "half": half, "xt_r": xt_r, "tok": tok}

        def emit_router(st):
            """mm1 -> silu -> (low interleaved) -> mm2"""
            xt_r = st["xt_r"]
            h_ps = ps_h.tile([128, RC, TT], F32, tag="h")
            for c2 in range(RC):
                for c in range(HC):
                    nc.tensor.matmul(
                        out=h_ps[:, c2, :],
                        lhsT=w1_r[:, c, ts(c2, 128)],
                        rhs=xt_r[:, c, :],
                        start=(c == 0),
                        stop=(c == HC - 1),
                    )
            # silu(z) = z * sigmoid(z), z = h + b1: ACT computes sigmoid and
            # z (identity+bias), Pool multiplies (keeps DVE free)
            sg_sb = ht_p.tile([128, RC, TT], F32, tag="sg")
            hb_sb = ht_p.tile([128, RC, TT], F32, tag="hb")
            ht_sb = ht_p.tile([128, RC, TT], F32, tag="ht")
            for c2 in range(RC):
                nc.scalar.activation(
                    out=sg_sb[:, c2, :], in_=h_ps[:, c2, :],
                    func=AF.Sigmoid, bias=b1_sb[:, c2 : c2 + 1],
                )
                nc.scalar.activation(
                    out=hb_sb[:, c2, :], in_=h_ps[:, c2, :],
                    func=AF.Identity, bias=b1_sb[:, c2 : c2 + 1],
                )
            nc.gpsimd.tensor_tensor(out=ht_sb, in0=hb_sb, in1=sg_sb, op=ALU.mult)

            # lowT[(e,r), t] = (x @ Acat)^T (f32r, N=512)
            low_ps = ps_lo.tile([ER, TT], F32, tag="lo")
            for c in range(HC):
                nc.tensor.matmul(
                    out=low_ps,
                    lhsT=acat_r[:, c, :, :],
                    rhs=xt_r[:, c, :],
                    start=(c == 0),
                    stop=(c == HC - 1),
                )
            # wl bank: lg f32 in [:, 0:20], wrt bf16 in f32-cols [128:384]
            wl = ps_wl.tile([128, 512], F32, tag="wl")
            lg_ps = wl[:, 0 : JJ * E].rearrange("p (j e) -> p j e", j=JJ)
            # router mm2 (token-major logits), exact fp32, b2 added on DVE
            for j in range(JJ):
                for c2 in range(RC):
                    nc.tensor.matmul(
                        out=lg_ps[:, j, :],
                        lhsT=ht_sb[:, c2, ts(j, 128)],
                        rhs=w2_sb[:, c2, :],
                        start=(c2 == 0),
                        stop=(c2 == RC - 1),
                    )
            st["low_ps"], st["wl"], st["lg_ps"] = low_ps, wl, lg_ps

        def emit_weights(st):
            """j-merged top-2 + renormalized weights:
            w[e] = sigmoid(2*lg_e - m1 - m2) * [lg_e >= m2], expanded over r."""
            lg = small_p.tile([128, JJ, E], F32, tag="lg")
            nc.vector.tensor_tensor(
                out=lg, in0=st["lg_ps"],
                in1=b2_rep.unsqueeze(1).to_broadcast((128, JJ, E)), op=ALU.add,
            )
            m1 = small_p.tile([128, JJ], F32, tag="m1")
            nc.vector.tensor_reduce(out=m1, in_=lg, axis=AX.X, op=ALU.max)
            mask1 = small_p.tile([128, JJ, E], F32, tag="mask1")
            nc.vector.tensor_tensor(
                out=mask1, in0=lg, in1=m1.unsqueeze(2).to_broadcast((128, JJ, E)),
                op=ALU.is_equal,
            )
            masked = small_p.tile([128, JJ, E], F32, tag="masked")
            nc.vector.scalar_tensor_tensor(
                out=masked, in0=mask1, scalar=-1e30, in1=lg,
                op0=ALU.mult, op1=ALU.add,
            )
            m2 = small_p.tile([128, JJ], F32, tag="m2")
            nc.vector.tensor_reduce(out=m2, in_=masked, axis=AX.X, op=ALU.max)
            s2 = small_p.tile([128, JJ], F32, tag="s2")
            nc.vector.tensor_tensor(out=s2, in0=m1, in1=m2, op=ALU.add)
            argt = small_p.tile([128, JJ, E], F32, tag="argt")
            nc.vector.scalar_tensor_tensor(
                out=argt, in0=lg, scalar=2.0,
                in1=s2.unsqueeze(2).to_broadcast((128, JJ, E)),
                op0=ALU.mult, op1=ALU.subtract,
            )
            sig = small_p.tile([128, JJ, E], F32, tag="sig")
            nc.scalar.activation(out=sig, in_=argt, func=AF.Sigmoid)
            mge = small_p.tile([128, JJ, E], F32, tag="mge")
            nc.vector.tensor_tensor(
                out=mge, in0=lg, in1=m2.unsqueeze(2).to_broadcast((128, JJ, E)),
                op=ALU.is_ge,
            )
            # fused weight + expansion over r: w_exp[t, j, e, r] = sig*mge
            w_exp = small_p.tile([128, JJ, E, R], BF16, tag="w_exp")
            nc.vector.tensor_tensor(
                out=w_exp,
                in0=sig.unsqueeze(3).to_broadcast((128, JJ, E, R)),
                in1=mge.unsqueeze(3).to_broadcast((128, JJ, E, R)),
                op=ALU.mult,
            )
            st["w_exp"] = w_exp

        def emit_m(st):
            """wT transpose (into wl bank, bf16) + weighted-low"""
            # bf16 view of wl f32-cols [128:384] = 512 bf16 cols
            wrt_ps = st["wl"][:, 128:384].bitcast(BF16)[0:ER, :].rearrange(
                "p (j t) -> p j t", j=JJ
            )
            for j in range(JJ):
                nc.tensor.transpose(
                    out=wrt_ps[:, j, :],
                    in_=st["w_exp"][:, j, :, :].rearrange("p e r -> p (e r)"),
                    identity=ident_bf,
                )
            wrt_sb = lw_p.tile([ER, JJ, 128], BF16, tag="wrt_sb")
            nc.scalar.copy(out=wrt_sb, in_=wrt_ps)
            lw_sb = lw_p.tile([ER, TT], BF16)
            nc.vector.tensor_tensor(
                out=lw_sb,
                in0=st["low_ps"],
                in1=wrt_sb.rearrange("p j t -> p (j t)"),
                op=ALU.mult,
            )
            st["lw_sb"] = lw_sb

        def emit_b(st):
            """back half: delta matmuls (bf16) in 5 bank-aligned chunks of the
            flat (j, h) output, fused PSUM+base adds, store"""
            bo, lw_sb, tok = st["bo"], st["lw_sb"], st["tok"]
            bo_flat = bo.rearrange("p j h -> p (j h)")
            for c5, parts in enumerate(DELTA_CHUNKS):
                dl = ps_dl.tile([128, 512], F32, tag="dl")
                for j, h0, h1, off in parts:
                    nc.tensor.matmul(
                        out=dl[:, off : off + (h1 - h0)],
                        lhsT=lw_sb[:, ts(j, 128)],
                        rhs=bcat_bf[:, h0:h1],
                        start=True, stop=True,
                    )
                nc.vector.tensor_tensor(
                    out=bo_flat[:, c5 * 512 : (c5 + 1) * 512],
                    in0=dl,
                    in1=bo_flat[:, c5 * 512 : (c5 + 1) * 512],
                    op=ALU.add,
                )
            # store on the SWDGE (gpsimd) queue: keeps the waiting store off
            # the HWDGE FIFOs so it can't head-of-line block the next loads
            nc.gpsimd.dma_start(
                out=out_d[tok : tok + TT, :].rearrange("(j p) h -> p j h", p=128),
                in_=bo,
            )

        prev = None
        x2 = bo2 = None
        for i in range(ntiles):
            if i % 2 == 0:
                x2, bo2 = emit_loads(i // 2)
            st = emit_front(i, x2, bo2)
            emit_router(st)
            emit_weights(st)
            emit_m(st)
            if prev is not None:
                emit_b(prev)
            prev = st
        emit_b(prev)

    return nc


_CACHE = {}


def _get_nc(t_core=T_CORE, niter=1, timing_mode=False):
    key = (t_core, niter, timing_mode)
    if key not in _CACHE:
        nc = build_kernel(t_core, niter, timing_mode)
        nc.finalize()
        _CACHE[key] = nc
    return _CACHE[key]


def kernel(x, base_output, W1, b1, W2, b2, A, Bm):
    x = np.ascontiguousarray(np.asarray(x), dtype=np.float32)
    base_output = np.ascontiguousarray(np.asarray(base_output), dtype=np.float32)
    W1 = np.ascontiguousarray(np.asarray(W1), dtype=np.float32)
    b1 = np.ascontiguousarray(np.asarray(b1), dtype=np.float32)
    W2 = np.ascontiguousarray(np.asarray(W2), dtype=np.float32)
    b2 = np.ascontiguousarray(np.asarray(b2), dtype=np.float32)
    A = np.ascontiguousarray(np.asarray(A), dtype=np.float32)
    Bm = np.ascontiguousarray(np.asarray(Bm), dtype=np.float32)

    B, S, _ = x.shape
    assert B * S == N_CORES * T_CORE
    xs = x.reshape(N_CORES, T_CORE, H)
    bs = base_output.reshape(N_CORES, T_CORE, H)

    nc = _get_nc()
    in_maps = [
        {
            "x": np.ascontiguousarray(xs[i]),
            "base": np.ascontiguousarray(bs[i]),
            "W1": W1, "b1": b1, "W2": W2, "b2": b2, "A": A, "Bm": Bm,
        }
        for i in range(N_CORES)
    ]
    res = run_bass_kernel_spmd(nc, in_maps, list(range(N_CORES))).results
    out = np.stack([res[i]["out"] for i in range(N_CORES)], axis=0)
    return out.reshape(B, S, H).astype(np.float32)


# revision 25
# speedup vs baseline: 1.2479x; 1.1224x over previous
"""MoLoRA (mixture of LoRA experts with top-2 routing) Trainium2 Bass kernel.

Math (per token t, hidden H=640, experts E=5, rank R=8, router hidden 256):
  h      = silu(x @ W1 + b1)                 [T, 256]
  logits = h @ W2 + b2                       [T, 5]
  top-2 of softmax(logits), renormalized  == softmax over the top-2 logits:
     w[t, e] = sigmoid(2*l_e - m1 - m2) * [l_e >= m2]   (m1/m2 = top-2 logits)
  low    = x @ Acat                          [T, 40]   (Acat[h,(e,r)] = A[e,h,r])
  delta  = (low * w_expanded) @ (Bcat * 2)   [T, 640]  (Bcat[(e,r),h] = Bm[e,r,h])
  out    = base_output + delta

mm1/low run in float32r (fast-fp32 PE mode, full rate at N>=256) off exact
fp32 PE transposes of x; mm2 is exact fp32 (tiny); the delta path is bf16.
Tiles are 512 tokens to amortize per-instruction overhead.  Sharding:
data-parallel over 8 NeuronCores (4096 tokens each), params replicated.
"""

import numpy as np
from contextlib import ExitStack

import concourse.bass as bass
import concourse.tile as tile
from concourse import bacc
from concourse import mybir
from concourse.bass import ts
from concourse.masks import make_identity
from concourse.bass_utils import run_bass_kernel_spmd

F32 = mybir.dt.float32
F32R = mybir.dt.float32r
BF16 = mybir.dt.bfloat16
AF = mybir.ActivationFunctionType
ALU = mybir.AluOpType
AX = mybir.AxisListType

H = 640          # hidden
E = 5            # experts
R = 8            # lora rank
ER = E * R       # 40
RH = 256         # router hidden
HC = H // 128    # 5 h-chunks
RC = RH // 128   # 2 router-hidden chunks
SCALING = 16.0 / R
N_CORES = 8
T_CORE = 4096    # tokens per core (32768 / 8)
TT = 512         # token tile (4 j-halves of 128)
JJ = TT // 128   # 4

# delta is computed in 5 PSUM-bank-aligned 512-column chunks of the flat
# (j, h) output: chunk -> list of (j, h0, h1, dl_offset)
DELTA_CHUNKS = []
for _c5 in range(5):
    _g0, _g1 = _c5 * 512, (_c5 + 1) * 512
    _parts = []
    for _j in range(JJ):
        _a, _b = max(_g0, _j * H), min(_g1, (_j + 1) * H)
        if _a < _b:
            _parts.append((_j, _a - _j * H, _b - _j * H, _a - _g0))
    DELTA_CHUNKS.append(_parts)


def build_kernel(t_core=T_CORE, niter=1, timing_mode=False):
    assert t_core % TT == 0
    ntiles = t_core // TT
    nc = bacc.Bacc()

    if timing_mode:
        # big tensors stay on-device (uninitialized DRAM) so per-call wall
        # time isn't dominated by the axon host transfer; HBM traffic is
        # identical to the real kernel.
        x_d = nc.dram_tensor("x_int", [t_core, H], F32)[:, :]
        base_d = nc.dram_tensor("base_int", [t_core, H], F32)[:, :]
        out_d = nc.dram_tensor("out_int", [t_core, H], F32)[:, :]
        dummy_d = nc.declare_dram_parameter("dummy_out", [1, 4], F32, isOutput=True)
    else:
        x_d = nc.declare_dram_parameter("x", [t_core, H], F32, isOutput=False)
        base_d = nc.declare_dram_parameter("base", [t_core, H], F32, isOutput=False)
        out_d = nc.declare_dram_parameter("out", [t_core, H], F32, isOutput=True)
        dummy_d = None
    w1_d = nc.declare_dram_parameter("W1", [H, RH], F32, isOutput=False)
    b1_d = nc.declare_dram_parameter("b1", [RH], F32, isOutput=False)
    w2_d = nc.declare_dram_parameter("W2", [RH, E], F32, isOutput=False)
    b2_d = nc.declare_dram_parameter("b2", [E], F32, isOutput=False)
    a_d = nc.declare_dram_parameter("A", [E, H, R], F32, isOutput=False)
    bm_d = nc.declare_dram_parameter("Bm", [E, R, H], F32, isOutput=False)

    with ExitStack() as ctx:
        tc = ctx.enter_context(tile.TileContext(nc))
        const = ctx.enter_context(tc.tile_pool(name="const", bufs=1))
        xin_p = ctx.enter_context(tc.tile_pool(name="xin", bufs=2))
        bout_p = ctx.enter_context(tc.tile_pool(name="bout", bufs=2))
        xt_p = ctx.enter_context(tc.tile_pool(name="xt", bufs=2))
        ht_p = ctx.enter_context(tc.tile_pool(name="ht", bufs=2))
        small_p = ctx.enter_context(tc.tile_pool(name="small", bufs=2))
        lw_p = ctx.enter_context(tc.tile_pool(name="lw", bufs=2))
        # PSUM budget (8 banks of 2KB):
        #  ps_xtp [128, 512] f32 bufs=2                  -> 2 banks
        #  ps_h   [128, 2, 512] f32 bufs=1               -> 2 banks
        #  ps_lo  [40, 512] f32 bufs=1                   -> 1 bank
        #  ps_wl  [128, 512] f32 bufs=1 (lg + wrt bf16)  -> 1 bank
        #  ps_dl  [128, 512] f32 bufs=2                  -> 2 banks
        ps_xtp = ctx.enter_context(tc.tile_pool(name="ps_xtp", bufs=2, space="PSUM"))
        ps_h = ctx.enter_context(tc.tile_pool(name="ps_h", bufs=1, space="PSUM"))
        ps_lo = ctx.enter_context(tc.tile_pool(name="ps_lo", bufs=1, space="PSUM"))
        ps_wl = ctx.enter_context(tc.tile_pool(name="ps_wl", bufs=1, space="PSUM"))
        ps_dl = ctx.enter_context(tc.tile_pool(name="ps_dl", bufs=2, space="PSUM"))

        # ---- constants / replicated params ----
        ident = const.tile([128, 128], F32)
        make_identity(nc, ident)
        ident_bf = const.tile([128, 128], BF16)
        nc.vector.tensor_copy(out=ident_bf, in_=ident)

        w1_sb = const.tile([128, HC, RH], F32)
        nc.gpsimd.dma_start(out=w1_sb, in_=w1_d.rearrange("(c p) m -> p c m", p=128))
        w1_r = const.tile([128, HC, RH], F32R)
        nc.vector.tensor_copy(out=w1_r, in_=w1_sb)
        b1_sb = const.tile([128, RC], F32)
        nc.gpsimd.dma_start(out=b1_sb, in_=b1_d.rearrange("(c p) -> p c", p=128))
        w2_sb = const.tile([128, RC, E], F32)
        nc.gpsimd.dma_start(out=w2_sb, in_=w2_d.rearrange("(c p) e -> p c e", p=128))
        # b2 replicated to all partitions (added on DVE, not via PE)
        b2_rep = const.tile([128, E], F32)
        nc.gpsimd.dma_start(
            out=b2_rep, in_=b2_d[:].unsqueeze(0).to_broadcast((128, E))
        )
        # LoRA params concatenated over (e, r): index m = e*R + r.
        acat_sb = const.tile([128, HC, E, R], F32)
        for e in range(E):
            for c in range(HC):
                nc.gpsimd.dma_start(
                    out=acat_sb[:, c, e, :],
                    in_=a_d[e, c * 128 : (c + 1) * 128, :],
                )
        acat_r = const.tile([128, HC, E, R], F32R)
        nc.vector.tensor_copy(out=acat_r, in_=acat_sb)
        bcat_sb = const.tile([ER, H], F32)
        for e in range(E):
            nc.gpsimd.dma_start(out=bcat_sb[e * R : (e + 1) * R, :], in_=bm_d[e, :, :])
        # LoRA SCALING (=2.0) folded into Bcat here; delta path is bf16.
        bcat_bf = const.tile([ER, H], BF16)
        nc.vector.tensor_scalar(
            out=bcat_bf, in0=bcat_sb, scalar1=float(SCALING), scalar2=None,
            op0=ALU.mult,
        )

        if dummy_d is not None:
            dnm = const.tile([1, 4], F32)
            nc.vector.memset(dnm, 1.0)
            nc.sync.dma_start(out=dummy_d[:, :], in_=dnm)

        loop_ctx = tc.For_i(0, niter, 1) if niter > 1 else None
        if loop_ctx is not None:
            ctx.enter_context(loop_ctx)

        def emit_loads(ip):
            """one 1024-token (2-tile) load pair on SP/ACT HWDGE"""
            tok = ip * 2 * TT
            x2 = xin_p.tile([128, 2 * JJ, H], F32)
            nc.sync.dma_start(
                out=x2,
                in_=x_d[tok : tok + 2 * TT, :].rearrange("(j p) h -> p j h", p=128),
            )
            bo2 = bout_p.tile([128, 2 * JJ, H], F32)
            with tc.high_priority():
                nc.scalar.dma_start(
                    out=bo2,
                    in_=base_d[tok : tok + 2 * TT, :].rearrange(
                        "(j p) h -> p j h", p=128
                    ),
                )
            return x2, bo2

        def emit_front(i, x2, bo2):
            """xT transposes (exact fp32) + f32r copy, c-chunk rotated"""
            tok = i * TT
            half = i % 2
            x_nat = x2[:, half * JJ : (half + 1) * JJ, :]
            bo = bo2[:, half * JJ : (half + 1) * JJ, :]
            xt_r = xt_p.tile([128, HC, TT], F32R)
            for c in range(HC):
                xtp = ps_xtp.tile([128, TT], F32, tag="xtp")
                for tj in range(JJ):
                    nc.tensor.transpose(
                        out=xtp[:, ts(tj, 128)],
                        in_=x_nat[:, tj, ts(c, 128)],
                        identity=ident,
                    )
                nc.scalar.copy(out=xt_r[:, c, :], in_=xtp)
            return {"bo": bo, "bo2": bo2, "half": half, "xt_r": xt_r, "tok": tok}

        def emit_router(st):
            """mm1 -> silu -> (low interleaved) -> mm2"""
            xt_r = st["xt_r"]
            h_ps = ps_h.tile([128, RC, TT], F32, tag="h")
            for c2 in range(RC):
                for c in range(HC):
                    nc.tensor.matmul(
                        out=h_ps[:, c2, :],
                        lhsT=w1_r[:, c, ts(c2, 128)],
                        rhs=xt_r[:, c, :],
                        start=(c == 0),
                        stop=(c == HC - 1),
                    )
            # silu(z) = z * sigmoid(z), z = h + b1: ACT computes sigmoid and
            # z (identity+bias), Pool multiplies (keeps DVE free)
            sg_sb = ht_p.tile([128, RC, TT], F32, tag="sg")
            ht_sb = ht_p.tile([128, RC, TT], F32, tag="ht")
            for c2 in range(RC):
                nc.scalar.activation(
                    out=sg_sb[:, c2, :], in_=h_ps[:, c2, :],
                    func=AF.Sigmoid, bias=b1_sb[:, c2 : c2 + 1],
                )
                nc.vector.scalar_tensor_tensor(
                    out=ht_sb[:, c2, :], in0=h_ps[:, c2, :],
                    scalar=b1_sb[:, c2 : c2 + 1], in1=sg_sb[:, c2, :],
                    op0=ALU.add, op1=ALU.mult,
                )

            # lowT[(e,r), t] = (x @ Acat)^T (f32r, N=512)
            low_ps = ps_lo.tile([ER, TT], F32, tag="lo")
            for c in range(HC):
                nc.tensor.matmul(
                    out=low_ps,
                    lhsT=acat_r[:, c, :, :],
                    rhs=xt_r[:, c, :],
                    start=(c == 0),
                    stop=(c == HC - 1),
                )
            # wl bank: lg f32 in [:, 0:20], wrt bf16 in f32-cols [128:384]
            wl = ps_wl.tile([128, 512], F32, tag="wl")
            lg_ps = wl[:, 0 : JJ * E].rearrange("p (j e) -> p j e", j=JJ)
            # router mm2 (token-major logits), exact fp32, b2 added on DVE
            for j in range(JJ):
                for c2 in range(RC):
                    nc.tensor.matmul(
                        out=lg_ps[:, j, :],
                        lhsT=ht_sb[:, c2, ts(j, 128)],
                        rhs=w2_sb[:, c2, :],
                        start=(c2 == 0),
                        stop=(c2 == RC - 1),
                    )
            st["low_ps"], st["wl"], st["lg_ps"] = low_ps, wl, lg_ps

        def emit_weights(st):
            """j-merged top-2 + renormalized weights:
            w[e] = sigmoid(2*lg_e - m1 - m2) * [lg_e >= m2], expanded over r."""
            lg = small_p.tile([128, JJ, E], F32, tag="lg")
            nc.vector.tensor_tensor(
                out=lg, in0=st["lg_ps"],
                in1=b2_rep.unsqueeze(1).to_broadcast((128, JJ, E)), op=ALU.add,
            )
            m1 = small_p.tile([128, JJ], F32, tag="m1")
            nc.vector.tensor_reduce(out=m1, in_=lg, axis=AX.X, op=ALU.max)
            mask1 = small_p.tile([128, JJ, E], F32, tag="mask1")
            nc.vector.tensor_tensor(
                out=mask1, in0=lg, in1=m1.unsqueeze(2).to_broadcast((128, JJ, E)),
                op=ALU.is_equal,
            )
            masked = small_p.tile([128, JJ, E], F32, tag="masked")
            nc.vector.scalar_tensor_tensor(
                out=masked, in0=mask1, scalar=-1e30, in1=lg,
                op0=ALU.mult, op1=ALU.add,
            )
            m2 = small_p.tile([128, JJ], F32, tag="m2")
            nc.vector.tensor_reduce(out=m2, in_=masked, axis=AX.X, op=ALU.max)
            s2 = small_p.tile([128, JJ], F32, tag="s2")
            nc.vector.tensor_tensor(out=s2, in0=m1, in1=m2, op=ALU.add)
            argt = small_p.tile([128, JJ, E], F32, tag="argt")
            nc.vector.scalar_tensor_tensor(
                out=argt, in0=lg, scalar=2.0,
                in1=s2.unsqueeze(2).to_broadcast((128, JJ, E)),
                op0=ALU.mult, op1=ALU.subtract,
            )
            sig = small_p.tile([128, JJ, E], F32, tag="sig")
            nc.scalar.activation(out=sig, in_=argt, func=AF.Sigmoid)
            mge = small_p.tile([128, JJ, E], F32, tag="mge")
            nc.vector.tensor_tensor(
                out=mge, in0=lg, in1=m2.unsqueeze(2).to_broadcast((128, JJ, E)),
                op=ALU.is_ge,
            )
            # fused weight + expansion over r: w_exp[t, j, e, r] = sig*mge
            w_exp = small_p.tile([128, JJ, E, R], BF16, tag="w_exp")
            nc.vector.tensor_tensor(
                out=w_exp,
                in0=sig.unsqueeze(3).to_broadcast((128, JJ, E, R)),
                in1=mge.unsqueeze(3).to_broadcast((128, JJ, E, R)),
                op=ALU.mult,
            )
            st["w_exp"] = w_exp

        def emit_m(st):
            """wT transpose (into wl bank, bf16) + weighted-low"""
            # bf16 view of wl f32-cols [128:384] = 512 bf16 cols
            wrt_ps = st["wl"][:, 128:384].bitcast(BF16)[0:ER, :].rearrange(
                "p (j t) -> p j t", j=JJ
            )
            for j in range(JJ):
                nc.tensor.transpose(
                    out=wrt_ps[:, j, :],
                    in_=st["w_exp"][:, j, :, :].rearrange("p e r -> p (e r)"),
                    identity=ident_bf,
                )
            wrt_sb = lw_p.tile([ER, JJ, 128], BF16, tag="wrt_sb")
            nc.scalar.copy(out=wrt_sb, in_=wrt_ps)
            lw_sb = lw_p.tile([ER, TT], BF16)
            nc.vector.tensor_tensor(
                out=lw_sb,
                in0=st["low_ps"],
                in1=wrt_sb.rearrange("p j t -> p (j t)"),
                op=ALU.mult,
            )
            st["lw_sb"] = lw_sb

        def emit_b(st):
            """back half: delta matmuls (bf16) in 5 bank-aligned chunks of the
            flat (j, h) output, fused PSUM+base adds, store"""
            bo, lw_sb, tok = st["bo"], st["lw_sb"], st["tok"]
            bo_flat = bo.rearrange("p j h -> p (j h)")
            for c5, parts in enumerate(DELTA_CHUNKS):
                dl = ps_dl.tile([128, 512], F32, tag="dl")
                for j, h0, h1, off in parts:
                    nc.tensor.matmul(
                        out=dl[:, off : off + (h1 - h0)],
                        lhsT=lw_sb[:, ts(j, 128)],
                        rhs=bcat_bf[:, h0:h1],
                        start=True, stop=True,
                    )
                nc.vector.tensor_tensor(
                    out=bo_flat[:, c5 * 512 : (c5 + 1) * 512],
                    in0=dl,
                    in1=bo_flat[:, c5 * 512 : (c5 + 1) * 512],
                    op=ALU.add,
                )
            # store on the SWDGE (gpsimd) queue: keeps the waiting store off
            # the HWDGE FIFOs so it can't head-of-line block the next loads
            nc.gpsimd.dma_start(
                out=out_d[tok : tok + TT, :].rearrange("(j p) h -> p j h", p=128),
                in_=bo,
            )

        prev = None
        x2 = bo2 = None
        for i in range(ntiles):
            if i % 2 == 0:
                x2, bo2 = emit_loads(i // 2)
            st = emit_front(i, x2, bo2)
            emit_router(st)
            emit_weights(st)
            emit_m(st)
            if prev is not None:
                emit_b(prev)
            prev = st
        emit_b(prev)

    return nc


_CACHE = {}


def _get_nc(t_core=T_CORE, niter=1, timing_mode=False):
    key = (t_core, niter, timing_mode)
    if key not in _CACHE:
        nc = build_kernel(t_core, niter, timing_mode)
        nc.finalize()
        _CACHE[key] = nc
    return _CACHE[key]


def kernel(x, base_output, W1, b1, W2, b2, A, Bm):
    x = np.ascontiguousarray(np.asarray(x), dtype=np.float32)
    base_output = np.ascontiguousarray(np.asarray(base_output), dtype=np.float32)
    W1 = np.ascontiguousarray(np.asarray(W1), dtype=np.float32)
    b1 = np.ascontiguousarray(np.asarray(b1), dtype=np.float32)
    W2 = np.ascontiguousarray(np.asarray(W2), dtype=np.float32)
    b2 = np.ascontiguousarray(np.asarray(b2), dtype=np.float32)
    A = np.ascontiguousarray(np.asarray(A), dtype=np.float32)
    Bm = np.ascontiguousarray(np.asarray(Bm), dtype=np.float32)

    B, S, _ = x.shape
    assert B * S == N_CORES * T_CORE
    xs = x.reshape(N_CORES, T_CORE, H)
    bs = base_output.reshape(N_CORES, T_CORE, H)

    nc = _get_nc()
    in_maps = [
        {
            "x": np.ascontiguousarray(xs[i]),
            "base": np.ascontiguousarray(bs[i]),
            "W1": W1, "b1": b1, "W2": W2, "b2": b2, "A": A, "Bm": Bm,
        }
        for i in range(N_CORES)
    ]
    res = run_bass_kernel_spmd(nc, in_maps, list(range(N_CORES))).results
    out = np.stack([res[i]["out"] for i in range(N_CORES)], axis=0)
    return out.reshape(B, S, H).astype(np.float32)


# revision 26
# speedup vs baseline: 1.2623x; 1.0115x over previous
"""MoLoRA (mixture of LoRA experts with top-2 routing) Trainium2 Bass kernel.

Math (per token t, hidden H=640, experts E=5, rank R=8, router hidden 256):
  h      = silu(x @ W1 + b1)                 [T, 256]
  logits = h @ W2 + b2                       [T, 5]
  top-2 of softmax(logits), renormalized  == softmax over the top-2 logits:
     w[t, e] = sigmoid(2*l_e - m1 - m2) * [l_e >= m2]   (m1/m2 = top-2 logits)
  low    = x @ Acat                          [T, 40]   (Acat[h,(e,r)] = A[e,h,r])
  delta  = (low * w_expanded) @ (Bcat * 2)   [T, 640]  (Bcat[(e,r),h] = Bm[e,r,h])
  out    = base_output + delta

mm1/low run in float32r (fast-fp32 PE mode, full rate at N>=256) off exact
fp32 PE transposes of x; mm2 is exact fp32 (tiny); the delta path is bf16.
Tiles are 512 tokens to amortize per-instruction overhead.  Sharding:
data-parallel over 8 NeuronCores (4096 tokens each), params replicated.
"""

import numpy as np
from contextlib import ExitStack

import concourse.bass as bass
import concourse.tile as tile
from concourse import bacc
from concourse import mybir
from concourse.bass import ts
from concourse.masks import make_identity
from concourse.bass_utils import run_bass_kernel_spmd

F32 = mybir.dt.float32
F32R = mybir.dt.float32r
BF16 = mybir.dt.bfloat16
AF = mybir.ActivationFunctionType
ALU = mybir.AluOpType
AX = mybir.AxisListType

H = 640          # hidden
E = 5            # experts
R = 8            # lora rank
ER = E * R       # 40
RH = 256         # router hidden
HC = H // 128    # 5 h-chunks
RC = RH // 128   # 2 router-hidden chunks
SCALING = 16.0 / R
N_CORES = 8
T_CORE = 4096    # tokens per core (32768 / 8)
TT = 512         # token tile (4 j-halves of 128)
JJ = TT // 128   # 4

# delta is computed in 5 PSUM-bank-aligned 512-column chunks of the flat
# (j, h) output: chunk -> list of (j, h0, h1, dl_offset)
DELTA_CHUNKS = []
for _c5 in range(5):
    _g0, _g1 = _c5 * 512, (_c5 + 1) * 512
    _parts = []
    for _j in range(JJ):
        _a, _b = max(_g0, _j * H), min(_g1, (_j + 1) * H)
        if _a < _b:
            _parts.append((_j, _a - _j * H, _b - _j * H, _a - _g0))
    DELTA_CHUNKS.append(_parts)


def build_kernel(t_core=T_CORE, niter=1, timing_mode=False):
    assert t_core % TT == 0
    ntiles = t_core // TT
    nc = bacc.Bacc()

    if timing_mode:
        # big tensors stay on-device (uninitialized DRAM) so per-call wall
        # time isn't dominated by the axon host transfer; HBM traffic is
        # identical to the real kernel.
        x_d = nc.dram_tensor("x_int", [t_core, H], F32)[:, :]
        base_d = nc.dram_tensor("base_int", [t_core, H], F32)[:, :]
        out_d = nc.dram_tensor("out_int", [t_core, H], F32)[:, :]
        dummy_d = nc.declare_dram_parameter("dummy_out", [1, 4], F32, isOutput=True)
    else:
        x_d = nc.declare_dram_parameter("x", [t_core, H], F32, isOutput=False)
        base_d = nc.declare_dram_parameter("base", [t_core, H], F32, isOutput=False)
        out_d = nc.declare_dram_parameter("out", [t_core, H], F32, isOutput=True)
        dummy_d = None
    w1_d = nc.declare_dram_parameter("W1", [H, RH], F32, isOutput=False)
    b1_d = nc.declare_dram_parameter("b1", [RH], F32, isOutput=False)
    w2_d = nc.declare_dram_parameter("W2", [RH, E], F32, isOutput=False)
    b2_d = nc.declare_dram_parameter("b2", [E], F32, isOutput=False)
    a_d = nc.declare_dram_parameter("A", [E, H, R], F32, isOutput=False)
    bm_d = nc.declare_dram_parameter("Bm", [E, R, H], F32, isOutput=False)

    with ExitStack() as ctx:
        tc = ctx.enter_context(tile.TileContext(nc))
        const = ctx.enter_context(tc.tile_pool(name="const", bufs=1))
        xin_p = ctx.enter_context(tc.tile_pool(name="xin", bufs=2))
        bout_p = ctx.enter_context(tc.tile_pool(name="bout", bufs=2))
        xt_p = ctx.enter_context(tc.tile_pool(name="xt", bufs=2))
        ht_p = ctx.enter_context(tc.tile_pool(name="ht", bufs=2))
        small_p = ctx.enter_context(tc.tile_pool(name="small", bufs=2))
        lw_p = ctx.enter_context(tc.tile_pool(name="lw", bufs=2))
        # PSUM budget (8 banks of 2KB):
        #  ps_xtp [128, 512] f32 bufs=2                  -> 2 banks
        #  ps_h   [128, 2, 512] f32 bufs=1               -> 2 banks
        #  ps_lo  [40, 512] f32 bufs=1                   -> 1 bank
        #  ps_wl  [128, 512] f32 bufs=1 (lg + wrt bf16)  -> 1 bank
        #  ps_dl  [128, 512] f32 bufs=2                  -> 2 banks
        ps_xtp = ctx.enter_context(tc.tile_pool(name="ps_xtp", bufs=2, space="PSUM"))
        ps_h = ctx.enter_context(tc.tile_pool(name="ps_h", bufs=1, space="PSUM"))
        ps_lo = ctx.enter_context(tc.tile_pool(name="ps_lo", bufs=1, space="PSUM"))
        ps_wl = ctx.enter_context(tc.tile_pool(name="ps_wl", bufs=1, space="PSUM"))
        ps_dl = ctx.enter_context(tc.tile_pool(name="ps_dl", bufs=2, space="PSUM"))

        # ---- constants / replicated params ----
        ident = const.tile([128, 128], F32)
        make_identity(nc, ident)
        ident_bf = const.tile([128, 128], BF16)
        nc.vector.tensor_copy(out=ident_bf, in_=ident)

        w1_sb = const.tile([128, HC, RH], F32)
        nc.gpsimd.dma_start(out=w1_sb, in_=w1_d.rearrange("(c p) m -> p c m", p=128))
        w1_r = const.tile([128, HC, RH], F32R)
        nc.vector.tensor_copy(out=w1_r, in_=w1_sb)
        b1_sb = const.tile([128, RC], F32)
        nc.gpsimd.dma_start(out=b1_sb, in_=b1_d.rearrange("(c p) -> p c", p=128))
        w2_sb = const.tile([128, RC, E], F32)
        nc.gpsimd.dma_start(out=w2_sb, in_=w2_d.rearrange("(c p) e -> p c e", p=128))
        # b2 replicated to all partitions (added on DVE, not via PE)
        b2_rep = const.tile([128, E], F32)
        nc.gpsimd.dma_start(
            out=b2_rep, in_=b2_d[:].unsqueeze(0).to_broadcast((128, E))
        )
        # LoRA params concatenated over (e, r): index m = e*R + r.
        acat_sb = const.tile([128, HC, E, R], F32)
        for e in range(E):
            for c in range(HC):
                nc.gpsimd.dma_start(
                    out=acat_sb[:, c, e, :],
                    in_=a_d[e, c * 128 : (c + 1) * 128, :],
                )
        acat_r = const.tile([128, HC, E, R], F32R)
        nc.vector.tensor_copy(out=acat_r, in_=acat_sb)
        bcat_sb = const.tile([ER, H], F32)
        for e in range(E):
            nc.gpsimd.dma_start(out=bcat_sb[e * R : (e + 1) * R, :], in_=bm_d[e, :, :])
        # LoRA SCALING (=2.0) folded into Bcat here; delta path is bf16.
        bcat_bf = const.tile([ER, H], BF16)
        nc.vector.tensor_scalar(
            out=bcat_bf, in0=bcat_sb, scalar1=float(SCALING), scalar2=None,
            op0=ALU.mult,
        )

        if dummy_d is not None:
            dnm = const.tile([1, 4], F32)
            nc.vector.memset(dnm, 1.0)
            nc.sync.dma_start(out=dummy_d[:, :], in_=dnm)

        loop_ctx = tc.For_i(0, niter, 1) if niter > 1 else None
        if loop_ctx is not None:
            ctx.enter_context(loop_ctx)

        def emit_loads(ip):
            """one 1024-token (2-tile) load pair on SP/ACT HWDGE"""
            tok = ip * 2 * TT
            x2 = xin_p.tile([128, 2 * JJ, H], F32)
            nc.sync.dma_start(
                out=x2,
                in_=x_d[tok : tok + 2 * TT, :].rearrange("(j p) h -> p j h", p=128),
            )
            bo2 = bout_p.tile([128, 2 * JJ, H], F32)
            with tc.high_priority():
                nc.scalar.dma_start(
                    out=bo2,
                    in_=base_d[tok : tok + 2 * TT, :].rearrange(
                        "(j p) h -> p j h", p=128
                    ),
                )
            return x2, bo2

        def emit_front(i, x2, bo2):
            """xT transposes (exact fp32) + f32r copy, c-chunk rotated"""
            tok = i * TT
            half = i % 2
            x_nat = x2[:, half * JJ : (half + 1) * JJ, :]
            bo = bo2[:, half * JJ : (half + 1) * JJ, :]
            xt_r = xt_p.tile([128, HC, TT], F32R)
            for c in range(HC):
                xtp = ps_xtp.tile([128, TT], F32, tag="xtp")
                for tj in range(JJ):
                    nc.tensor.transpose(
                        out=xtp[:, ts(tj, 128)],
                        in_=x_nat[:, tj, ts(c, 128)],
                        identity=ident,
                    )
                nc.scalar.copy(out=xt_r[:, c, :], in_=xtp)
            return {"bo": bo, "bo2": bo2, "half": half, "xt_r": xt_r, "tok": tok}

        def emit_router(st):
            """mm1 -> silu -> (low interleaved) -> mm2"""
            xt_r = st["xt_r"]
            h_ps = ps_h.tile([128, RC, TT], F32, tag="h")
            for c2 in range(RC):
                for c in range(HC):
                    nc.tensor.matmul(
                        out=h_ps[:, c2, :],
                        lhsT=w1_r[:, c, ts(c2, 128)],
                        rhs=xt_r[:, c, :],
                        start=(c == 0),
                        stop=(c == HC - 1),
                    )
            # silu(z) = z * sigmoid(z), z = h + b1: ACT sigmoid, DVE fused
            # (h + b1) * sg in one stt
            sg_sb = ht_p.tile([128, RC, TT], F32, tag="sg")
            ht_sb = ht_p.tile([128, RC, TT], F32, tag="ht")
            for c2 in range(RC):
                nc.scalar.activation(
                    out=sg_sb[:, c2, :], in_=h_ps[:, c2, :],
                    func=AF.Sigmoid, bias=b1_sb[:, c2 : c2 + 1],
                )
                nc.vector.scalar_tensor_tensor(
                    out=ht_sb[:, c2, :], in0=h_ps[:, c2, :],
                    scalar=b1_sb[:, c2 : c2 + 1], in1=sg_sb[:, c2, :],
                    op0=ALU.add, op1=ALU.mult,
                )

            # lowT[(e,r), t] = (x @ Acat)^T (f32r, N=512)
            low_ps = ps_lo.tile([ER, TT], F32, tag="lo")
            for c in range(HC):
                nc.tensor.matmul(
                    out=low_ps,
                    lhsT=acat_r[:, c, :, :],
                    rhs=xt_r[:, c, :],
                    start=(c == 0),
                    stop=(c == HC - 1),
                )
            # wl bank: lg f32 in [:, 0:20], wrt bf16 in f32-cols [128:384]
            wl = ps_wl.tile([128, 512], F32, tag="wl")
            lg_ps = wl[:, 0 : JJ * E].rearrange("p (j e) -> p j e", j=JJ)
            # router mm2 (token-major logits), exact fp32, b2 added on DVE
            for j in range(JJ):
                for c2 in range(RC):
                    nc.tensor.matmul(
                        out=lg_ps[:, j, :],
                        lhsT=ht_sb[:, c2, ts(j, 128)],
                        rhs=w2_sb[:, c2, :],
                        start=(c2 == 0),
                        stop=(c2 == RC - 1),
                    )
            st["low_ps"], st["wl"], st["lg_ps"] = low_ps, wl, lg_ps

        def emit_weights(st):
            """j-merged top-2 + renormalized weights:
            w[e] = sigmoid(2*lg_e - m1 - m2) * [lg_e >= m2], expanded over r."""
            lg = small_p.tile([128, JJ, E], F32, tag="lg")
            nc.vector.tensor_tensor(
                out=lg, in0=st["lg_ps"],
                in1=b2_rep.unsqueeze(1).to_broadcast((128, JJ, E)), op=ALU.add,
            )
            m1 = small_p.tile([128, JJ], F32, tag="m1")
            nc.vector.tensor_reduce(out=m1, in_=lg, axis=AX.X, op=ALU.max)
            mask1 = small_p.tile([128, JJ, E], F32, tag="mask1")
            nc.vector.tensor_tensor(
                out=mask1, in0=lg, in1=m1.unsqueeze(2).to_broadcast((128, JJ, E)),
                op=ALU.is_equal,
            )
            masked = small_p.tile([128, JJ, E], F32, tag="masked")
            nc.vector.scalar_tensor_tensor(
                out=masked, in0=mask1, scalar=-1e30, in1=lg,
                op0=ALU.mult, op1=ALU.add,
            )
            m2 = small_p.tile([128, JJ], F32, tag="m2")
            nc.vector.tensor_reduce(out=m2, in_=masked, axis=AX.X, op=ALU.max)
            s2 = small_p.tile([128, JJ], F32, tag="s2")
            nc.vector.tensor_tensor(out=s2, in0=m1, in1=m2, op=ALU.add)
            argt = small_p.tile([128, JJ, E], F32, tag="argt")
            nc.vector.scalar_tensor_tensor(
                out=argt, in0=lg, scalar=2.0,
                in1=s2.unsqueeze(2).to_broadcast((128, JJ, E)),
                op0=ALU.mult, op1=ALU.subtract,
            )
            sig = small_p.tile([128, JJ, E], F32, tag="sig")
            nc.scalar.activation(out=sig, in_=argt, func=AF.Sigmoid)
            mge = small_p.tile([128, JJ, E], F32, tag="mge")
            nc.vector.tensor_tensor(
                out=mge, in0=lg, in1=m2.unsqueeze(2).to_broadcast((128, JJ, E)),
                op=ALU.is_ge,
            )
            # fused weight + expansion over r: w_exp[t, j, e, r] = sig*mge
            w_exp = small_p.tile([128, JJ, E, R], BF16, tag="w_exp")
            nc.vector.tensor_tensor(
                out=w_exp,
                in0=sig.unsqueeze(3).to_broadcast((128, JJ, E, R)),
                in1=mge.unsqueeze(3).to_broadcast((128, JJ, E, R)),
                op=ALU.mult,
            )
            st["w_exp"] = w_exp

        def emit_m(st):
            """wT transpose (into wl bank, bf16) + weighted-low"""
            # bf16 view of wl f32-cols [128:384] = 512 bf16 cols
            wrt_ps = st["wl"][:, 128:384].bitcast(BF16)[0:ER, :].rearrange(
                "p (j t) -> p j t", j=JJ
            )
            for j in range(JJ):
                nc.tensor.transpose(
                    out=wrt_ps[:, j, :],
                    in_=st["w_exp"][:, j, :, :].rearrange("p e r -> p (e r)"),
                    identity=ident_bf,
                )
            wrt_sb = lw_p.tile([ER, JJ, 128], BF16, tag="wrt_sb")
            nc.scalar.copy(out=wrt_sb, in_=wrt_ps)
            lw_sb = lw_p.tile([ER, TT], BF16)
            nc.vector.tensor_tensor(
                out=lw_sb,
                in0=st["low_ps"],
                in1=wrt_sb.rearrange("p j t -> p (j t)"),
                op=ALU.mult,
            )
            st["lw_sb"] = lw_sb

        def emit_b(st):
            """back half: delta matmuls (bf16) in 5 bank-aligned chunks of the
            flat (j, h) output, fused PSUM+base adds, store"""
            bo, lw_sb, tok = st["bo"], st["lw_sb"], st["tok"]
            bo_flat = bo.rearrange("p j h -> p (j h)")
            for c5, parts in enumerate(DELTA_CHUNKS):
                dl = ps_dl.tile([128, 512], F32, tag="dl")
                for j, h0, h1, off in parts:
                    nc.tensor.matmul(
                        out=dl[:, off : off + (h1 - h0)],
                        lhsT=lw_sb[:, ts(j, 128)],
                        rhs=bcat_bf[:, h0:h1],
                        start=True, stop=True,
                    )
                nc.vector.tensor_tensor(
                    out=bo_flat[:, c5 * 512 : (c5 + 1) * 512],
                    in0=dl,
                    in1=bo_flat[:, c5 * 512 : (c5 + 1) * 512],
                    op=ALU.add,
                )
            # store on the SWDGE (gpsimd) queue: keeps the waiting store off
            # the HWDGE FIFOs so it can't head-of-line block the next loads
            nc.gpsimd.dma_start(
                out=out_d[tok : tok + TT, :].rearrange("(j p) h -> p j h", p=128),
                in_=bo,
            )

        prev = None
        x2 = bo2 = None
        for i in range(ntiles):
            if i % 2 == 0:
                x2, bo2 = emit_loads(i // 2)
            st = emit_front(i, x2, bo2)
            emit_router(st)
            emit_weights(st)
            emit_m(st)
            if prev is not None:
                emit_b(prev)
            prev = st
        emit_b(prev)

    return nc


_CACHE = {}


def _get_nc(t_core=T_CORE, niter=1, timing_mode=False):
    key = (t_core, niter, timing_mode)
    if key not in _CACHE:
        nc = build_kernel(t_core, niter, timing_mode)
        nc.finalize()
        _CACHE[key] = nc
    return _CACHE[key]


def kernel(x, base_output, W1, b1, W2, b2, A, Bm):
    x = np.ascontiguousarray(np.asarray(x), dtype=np.float32)
    base_output = np.ascontiguousarray(np.asarray(base_output), dtype=np.float32)
    W1 = np.ascontiguousarray(np.asarray(W1), dtype=np.float32)
    b1 = np.ascontiguousarray(np.asarray(b1), dtype=np.float32)
    W2 = np.ascontiguousarray(np.asarray(W2), dtype=np.float32)
    b2 = np.ascontiguousarray(np.asarray(b2), dtype=np.float32)
    A = np.ascontiguousarray(np.asarray(A), dtype=np.float32)
    Bm = np.ascontiguousarray(np.asarray(Bm), dtype=np.float32)

    B, S, _ = x.shape
    assert B * S == N_CORES * T_CORE
    xs = x.reshape(N_CORES, T_CORE, H)
    bs = base_output.reshape(N_CORES, T_CORE, H)

    nc = _get_nc()
    in_maps = [
        {
            "x": np.ascontiguousarray(xs[i]),
            "base": np.ascontiguousarray(bs[i]),
            "W1": W1, "b1": b1, "W2": W2, "b2": b2, "A": A, "Bm": Bm,
        }
        for i in range(N_CORES)
    ]
    res = run_bass_kernel_spmd(nc, in_maps, list(range(N_CORES))).results
    out = np.stack([res[i]["out"] for i in range(N_CORES)], axis=0)
    return out.reshape(B, S, H).astype(np.float32)
